# revision 9
# baseline (speedup 1.0000x reference)
import sys

import numpy as np

sys.path.insert(0, "/opt/trn_rl_repo")

T_FULL = 2048
E = 1024
S = 256
LN_EPS = 1e-5
N_CORES = 8


def _fold_weights(inputs):
    """Host-side algebra: collapse the reference network into 3 matrices.

    Kernel state is the raw GLU output y_t plus scalar r_t = 1/sqrt(var+eps).
    Mean subtraction and gamma fold into centered matrices
    C = D(gamma)W - (1/S)*1*(gamma^T W); beta folds into additive constants:
        q_{t+1} = y_t @ Cg
        g_{t+1} = r_t * q_{t+1} + v_{t+1},  v = emb @ W_V + bias_v
        y_{t+1} = g_a * sigmoid(g_b)
        out_t   = r_t * (y_t @ Co) + b_out
    """
    f8 = lambda a: np.asarray(a, dtype=np.float64)
    Wc = f8(inputs["W_state_control"])
    Wi = f8(inputs["W_input_influence"])
    Wo = f8(inputs["W_output_shaper"])
    Wglu = f8(inputs["W_glu"])
    b_glu = f8(inputs["b_glu"])
    gamma = f8(inputs["ln_gamma"])
    beta = f8(inputs["ln_beta"])
    We2s = f8(inputs["W_emb2state"])
    b_e2s = f8(inputs["b_emb2state"])
    Ws2o = f8(inputs["W_state2out"])
    b_s2o = f8(inputs["b_state2out"])

    Wcg = Wc @ Wglu                      # [S, 2S]
    Wig = Wi @ Wglu                      # [S, 2S]
    WO = Wo @ Ws2o                       # [S, E]

    ones = np.ones((S, 1))
    Cg = (gamma[:, None] * Wcg) - ones @ ((gamma @ Wcg)[None, :]) / S   # [S, 2S]
    Co = (gamma[:, None] * WO) - ones @ ((gamma @ WO)[None, :]) / S     # [S, E]

    W_V = We2s @ Wig                     # [E, 2S]
    bias_v = b_e2s @ Wig + b_glu + beta @ Wcg     # [2S]
    mwb = -(beta @ Wcg)                  # [2S]  (t=0 has no h-state, undo beta fold)
    b_out = b_s2o + beta @ WO            # [E]

    f4 = lambda a: np.ascontiguousarray(a, dtype=np.float32)
    return {
        "wv": f4(W_V),
        "cg": f4(Cg),
        "co": f4(Co),
        "vb": f4(np.broadcast_to(bias_v[None, :], (128, 2 * S))),
        "bo": f4(np.broadcast_to(b_out[None, :], (128, E))),
        "mwb": f4(mwb[None, :]),
    }


def build_nc(T):
    from concourse import bacc, bass, mybir
    from concourse.tile import TileContext

    dt = mybir.dt.float32
    Alu = mybir.AluOpType
    Act = mybir.ActivationFunctionType
    NT = T // 128  # number of 128-row time tiles

    nc = bacc.Bacc("TRN2", target_bir_lowering=False, debug=False)
    emb_d = nc.declare_dram_parameter("emb", [T, E], dt, isOutput=False)
    wv_d = nc.declare_dram_parameter("wv", [E, 2 * S], dt, isOutput=False)
    cg_d = nc.declare_dram_parameter("cg", [S, 2 * S], dt, isOutput=False)
    co_d = nc.declare_dram_parameter("co", [S, E], dt, isOutput=False)
    vb_d = nc.declare_dram_parameter("vb", [128, 2 * S], dt, isOutput=False)
    bo_d = nc.declare_dram_parameter("bo", [128, E], dt, isOutput=False)
    mwb_d = nc.declare_dram_parameter("mwb", [1, 2 * S], dt, isOutput=False)
    id_d = nc.declare_dram_parameter("ident", [128, 128], dt, isOutput=False)
    out_d = nc.declare_dram_parameter("out", [T, E], dt, isOutput=True)
    v_dram = nc.dram_tensor("v_dram", [T, 2 * S], dt)

    with TileContext(nc) as tc:
        with (
            tc.tile_pool(name="const", bufs=1) as cpool,
            tc.tile_pool(name="embp", bufs=2) as embp,
            tc.tile_pool(name="embtp", bufs=2) as embtp,
            tc.tile_pool(name="outp", bufs=2) as outp,
            tc.tile_pool(name="rowp", bufs=3) as rowp,
            tc.tile_pool(name="vrow", bufs=8) as vrowp,
            tc.tile_pool(name="vstage", bufs=2) as vstagep,
            tc.tile_pool(name="tiny", bufs=4) as tiny,
            tc.tile_pool(name="big_ps", bufs=2, space="PSUM") as big_ps,
            tc.tile_pool(name="tr_ps", bufs=2, space="PSUM") as tr_ps,
            tc.tile_pool(name="q_ps", bufs=2, space="PSUM") as q_ps_pool,
            tc.tile_pool(name="yt_ps", bufs=2, space="PSUM") as yt_ps_pool,
        ):
            # ---- persistent SBUF tensors ----
            wv_sb = cpool.tile([128, 8, 2 * S], dt, tag="wv")      # W_V row-chunks
            cg_sb = cpool.tile([128, 2, 2 * S], dt, tag="cg")
            co_sb = cpool.tile([128, 2, E], dt, tag="co")
            vb_sb = cpool.tile([128, 2 * S], dt, tag="vb")
            bo_sb = cpool.tile([128, E], dt, tag="bo")
            mwb_sb = cpool.tile([1, 2 * S], dt, tag="mwb")
            one_sb = cpool.tile([1, 1], dt, tag="one")
            id_sb = cpool.tile([128, 128], dt, tag="ident")
            YT_sb = cpool.tile([128, T, 2], dt, tag="YT")          # y_t[c*128+p] at [p,t,c]
            R_row = cpool.tile([1, T], dt, tag="R")                # r_t at [0,t]
            RS_row = cpool.tile([1, T], dt, tag="RS")              # 1/r_t at [0,t]

            for c in range(8):
                nc.sync.dma_start(out=wv_sb[:, c, :], in_=wv_d[c * 128:(c + 1) * 128, :])
            for c in range(2):
                nc.sync.dma_start(out=cg_sb[:, c, :], in_=cg_d[c * 128:(c + 1) * 128, :])
                nc.sync.dma_start(out=co_sb[:, c, :], in_=co_d[c * 128:(c + 1) * 128, :])
            nc.sync.dma_start(out=vb_sb[:], in_=vb_d[:])
            nc.sync.dma_start(out=bo_sb[:], in_=bo_d[:])
            nc.sync.dma_start(out=mwb_sb[:], in_=mwb_d[:])
            nc.sync.dma_start(out=id_sb[:], in_=id_d[:])
            nc.vector.memset(one_sb[:], 1.0)
            czero = cpool.tile([128, 1], dt, tag="czero")
            ceps = cpool.tile([128, 1], dt, tag="ceps")
            nc.vector.memset(czero[:], 0.0)
            nc.vector.memset(ceps[:], LN_EPS)
            nc.const_aps.aps[(dt, 0.0)] = czero[:]
            nc.const_aps.aps[(dt, LN_EPS)] = ceps[:]

            # ===== Phase 1: VT = (emb @ W_V + bias_v)^T, feature-major =====
            for i in range(NT):
                emb_t = embp.tile([128, E], dt, tag="emb")
                nc.sync.dma_start(out=emb_t[:], in_=emb_d[i * 128:(i + 1) * 128, :])
                embT = embtp.tile([128, 8, 128], dt, tag="embT")
                for c in range(8):
                    tp = tr_ps.tile([128, 128], dt, tag="tr")
                    nc.tensor.transpose(tp[:], emb_t[:, c * 128:(c + 1) * 128], id_sb[:])
                    nc.vector.tensor_copy(embT[:, c, :], tp[:])
                vps = big_ps.tile([128, 512], dt, tag="bigps")
                for ec in range(8):
                    nc.tensor.matmul(vps[:], lhsT=embT[:, ec, :], rhs=wv_sb[:, ec, :],
                                     start=(ec == 0), stop=(ec == 7))
                vrow_t = vstagep.tile([128, 2 * S], dt, tag="vstage")
                nc.vector.scalar_tensor_tensor(
                    out=vrow_t[:], in0=vps[:], scalar=1.0, in1=vb_sb[:],
                    op0=Alu.mult, op1=Alu.add)
                nc.sync.dma_start(out=v_dram[i * 128:(i + 1) * 128, :], in_=vrow_t[:])

            # ===== Phase 2: sequential scan =====
            # g_t (PSUM) = (1/r_{t-1}) * v_t + y_{t-1} @ Cg ; then
            # sigmoid/y-update apply r_{t-1} via the per-partition scale port.
            y_row = None
            for t in range(T):
                vsb = vrowp.tile([1, 2 * S], dt, tag="vrow")
                nc.sync.dma_start(out=vsb[:], in_=v_dram[t:t + 1, :])
                qp = q_ps_pool.tile([1, 2 * S], dt, tag="qps")
                if t == 0:
                    nc.tensor.matmul(qp[:], lhsT=one_sb[0:1, 0:1], rhs=vsb[:],
                                     start=True, stop=False)
                    nc.tensor.matmul(qp[:], lhsT=one_sb[0:1, 0:1], rhs=mwb_sb[:],
                                     start=False, stop=True)
                    r_scale = 1.0
                else:
                    # transpose y_{t-1} into YT[:, t-1, :] via K=1 matmuls
                    ytp = yt_ps_pool.tile([128, 2], dt, tag="ytps")
                    for c in range(2):
                        nc.tensor.matmul(ytp[:, c:c + 1], lhsT=y_row[0:1, c * 128:(c + 1) * 128],
                                         rhs=one_sb[0:1, 0:1], start=True, stop=True)
                    nc.vector.tensor_copy(YT_sb[:, t - 1, :], ytp[:])
                    # v-inject (independent of y): qp = (1/r_{t-1}) * v_t
                    nc.tensor.matmul(qp[:], lhsT=RS_row[0:1, t - 1:t], rhs=vsb[:],
                                     start=True, stop=False)
                    # qp += y_{t-1} @ Cg
                    for c in range(2):
                        nc.tensor.matmul(qp[:], lhsT=YT_sb[:, t - 1, c:c + 1], rhs=cg_sb[:, c, :],
                                         start=False, stop=(c == 1))
                    r_scale = R_row[0:1, t - 1:t]

                sig = rowp.tile([1, S], dt, tag="sig")
                nc.scalar.activation(sig[:], qp[0:1, S:2 * S], Act.Sigmoid, scale=r_scale)
                y_row = rowp.tile([1, S], dt, tag="y")
                sy = tiny.tile([1, 1], dt, tag="sy")
                nc.vector.scalar_tensor_tensor(
                    out=y_row[:], in0=qp[0:1, 0:S], scalar=r_scale, in1=sig[:],
                    op0=Alu.mult, op1=Alu.mult, accum_out=sy[:])
                # Sq' = sum((y/16)^2) = E[y^2]
                ysq = rowp.tile([1, S], dt, tag="ysq")
                sq_acc = tiny.tile([1, 1], dt, tag="sq")
                nc.scalar.activation(ysq[:], y_row[:], Act.Square, scale=1.0 / 16.0,
                                     accum_out=sq_acc[:])
                # 1/r = sqrt(E[y^2] - (Sy/256)^2 + eps) ; r = reciprocal
                sy2 = tiny.tile([1, 1], dt, tag="sy2")
                nc.scalar.activation(sy2[:], sy[:], Act.Square, scale=1.0 / S)
                t2 = tiny.tile([1, 1], dt, tag="t2")
                nc.scalar.activation(t2[:], sy2[:], Act.Identity, bias=sq_acc[:], scale=-1.0)
                nc.scalar.activation(RS_row[0:1, t:t + 1], t2[:], Act.Sqrt, bias=LN_EPS, scale=1.0)
                nc.vector.reciprocal(out=R_row[0:1, t:t + 1], in_=RS_row[0:1, t:t + 1])

            # final y_{T-1} transpose into YT
            ytp = yt_ps_pool.tile([128, 2], dt, tag="ytps")
            for c in range(2):
                nc.tensor.matmul(ytp[:, c:c + 1], lhsT=y_row[0:1, c * 128:(c + 1) * 128],
                                 rhs=one_sb[0:1, 0:1], start=True, stop=True)
            nc.vector.tensor_copy(YT_sb[:, T - 1, :], ytp[:])

            # ===== Phase 3: out = r * (y @ Co) + b_out =====
            for j in range(NT):
                rtp = yt_ps_pool.tile([128, 2], dt, tag="ytps")
                nc.tensor.matmul(rtp[:, 0:1], lhsT=R_row[0:1, j * 128:(j + 1) * 128],
                                 rhs=one_sb[0:1, 0:1], start=True, stop=True)
                rT = rowp.tile([128, 1], dt, tag="rT")
                nc.vector.tensor_copy(rT[:], rtp[:, 0:1])
                osb = outp.tile([128, E], dt, tag="osb")
                for nh in range(2):
                    ops = big_ps.tile([128, 512], dt, tag="bigps")
                    for c in range(2):
                        nc.tensor.matmul(
                            ops[:], lhsT=YT_sb[:, j * 128:(j + 1) * 128, c],
                            rhs=co_sb[:, c, nh * 512:(nh + 1) * 512],
                            start=(c == 0), stop=(c == 1))
                    nc.vector.scalar_tensor_tensor(
                        out=osb[:, nh * 512:(nh + 1) * 512], in0=ops[:],
                        scalar=rT[:, 0:1], in1=bo_sb[:, nh * 512:(nh + 1) * 512],
                        op0=Alu.mult, op1=Alu.add)
                nc.sync.dma_start(out=out_d[j * 128:(j + 1) * 128, :], in_=osb[:])

    nc.compile()
    return nc


_CACHE = {}


def _get_nc(T):
    if T not in _CACHE:
        _CACHE[T] = build_nc(T)
    return _CACHE[T]


def _run(inputs, trace=False):
    from concourse.bass_utils import run_bass_kernel_spmd

    emb = np.ascontiguousarray(inputs["embedded_tokens"], dtype=np.float32)
    B, T = emb.shape[0], emb.shape[1]
    folded = _fold_weights(inputs)
    nc = _get_nc(T)

    in_maps = []
    for b in range(B):
        m = {"emb": emb[b], "ident": np.eye(128, dtype=np.float32)}
        m.update(folded)
        in_maps.append(m)

    res = run_bass_kernel_spmd(nc, in_maps, core_ids=list(range(N_CORES)), trace=trace)
    out = np.stack([np.asarray(res.results[b]["out"]) for b in range(B)], axis=0)
    return out.astype(np.float32), res


def kernel(**inputs):
    out, _ = _run(inputs, trace=False)
    return out


# revision 16
# speedup vs baseline: 3.7787x; 3.7787x over previous
import sys

import numpy as np

sys.path.insert(0, "/opt/trn_rl_repo")

T_FULL = 2048
E = 1024
S = 256
LN_EPS = 1e-5
N_CORES = 8


def _fold_weights(inputs):
    """Host-side algebra: collapse the reference network into 3 matrices.

    Kernel state is the raw GLU output y_t plus scalar r_t = 1/sqrt(var+eps).
    Mean subtraction and gamma fold into centered matrices
    C = D(gamma)W - (1/S)*1*(gamma^T W); beta folds into additive constants:
        q_{t+1} = y_t @ Cg
        g_{t+1} = r_t * q_{t+1} + v_{t+1},  v = emb @ W_V + bias_v
        y_{t+1} = g_a * sigmoid(g_b)
        out_t   = r_t * (y_t @ Co) + b_out
    """
    f8 = lambda a: np.asarray(a, dtype=np.float64)
    Wc = f8(inputs["W_state_control"])
    Wi = f8(inputs["W_input_influence"])
    Wo = f8(inputs["W_output_shaper"])
    Wglu = f8(inputs["W_glu"])
    b_glu = f8(inputs["b_glu"])
    gamma = f8(inputs["ln_gamma"])
    beta = f8(inputs["ln_beta"])
    We2s = f8(inputs["W_emb2state"])
    b_e2s = f8(inputs["b_emb2state"])
    Ws2o = f8(inputs["W_state2out"])
    b_s2o = f8(inputs["b_state2out"])

    Wcg = Wc @ Wglu                      # [S, 2S]
    Wig = Wi @ Wglu                      # [S, 2S]
    WO = Wo @ Ws2o                       # [S, E]

    ones = np.ones((S, 1))
    Cg = (gamma[:, None] * Wcg) - ones @ ((gamma @ Wcg)[None, :]) / S   # [S, 2S]
    Co = (gamma[:, None] * WO) - ones @ ((gamma @ WO)[None, :]) / S     # [S, E]

    W_V = We2s @ Wig                     # [E, 2S]
    bias_v = b_e2s @ Wig + b_glu + beta @ Wcg     # [2S]
    mwb = -(beta @ Wcg)                  # [2S]  (t=0 has no h-state, undo beta fold)
    b_out = b_s2o + beta @ WO            # [E]

    import ml_dtypes
    f4 = lambda a: np.ascontiguousarray(a, dtype=np.float32)
    fb = lambda a: np.ascontiguousarray(a, dtype=ml_dtypes.bfloat16)
    return {
        "wv": fb(W_V),
        "cg": fb(Cg),
        "co": fb(Co),
        "vb": f4(np.broadcast_to(bias_v[None, :], (128, 2 * S))),
        "bo": f4(np.broadcast_to(b_out[None, :], (128, E))),
        "mwb": fb(mwb[None, :]),
    }


def build_nc(T, stats_dve=False, split_q=True, v_chunk=8, dbg_no_tr=False, dbg_no_act=False, dbg_no_stats=False):
    from concourse import bacc, bass, mybir
    from concourse.tile import TileContext

    dt = mybir.dt.float32
    bt = mybir.dt.bfloat16
    Alu = mybir.AluOpType
    Act = mybir.ActivationFunctionType
    NT = T // 128  # number of 128-row time tiles

    nc = bacc.Bacc("TRN2", target_bir_lowering=False, debug=False)
    emb_d = nc.declare_dram_parameter("emb", [T, E], dt, isOutput=False)
    wv_d = nc.declare_dram_parameter("wv", [E, 2 * S], bt, isOutput=False)
    cg_d = nc.declare_dram_parameter("cg", [S, 2 * S], bt, isOutput=False)
    co_d = nc.declare_dram_parameter("co", [S, E], bt, isOutput=False)
    vb_d = nc.declare_dram_parameter("vb", [128, 2 * S], dt, isOutput=False)
    bo_d = nc.declare_dram_parameter("bo", [128, E], dt, isOutput=False)
    mwb_d = nc.declare_dram_parameter("mwb", [1, 2 * S], bt, isOutput=False)
    id_d = nc.declare_dram_parameter("ident", [128, 128], dt, isOutput=False)
    out_d = nc.declare_dram_parameter("out", [T, E], dt, isOutput=True)
    v_dram = nc.dram_tensor("v_dram", [T, 2 * S], bt)

    with TileContext(nc) as tc:
        with (
            tc.tile_pool(name="const", bufs=1) as cpool,
            tc.tile_pool(name="embp", bufs=2) as embp,
            tc.tile_pool(name="embtp", bufs=2) as embtp,
            tc.tile_pool(name="outp", bufs=2) as outp,
            tc.tile_pool(name="rowp", bufs=3) as rowp,
            tc.tile_pool(name="vrow", bufs=3) as vrowp,
            tc.tile_pool(name="vstage", bufs=2) as vstagep,
            tc.tile_pool(name="tiny", bufs=4) as tiny,
            tc.tile_pool(name="big_ps", bufs=2, space="PSUM") as big_ps,
            tc.tile_pool(name="tr_ps", bufs=2, space="PSUM") as tr_ps,
            tc.tile_pool(name="q_ps", bufs=(1 if split_q else 2), space="PSUM") as q_ps_pool,
            tc.tile_pool(name="yt_ps", bufs=2, space="PSUM") as yt_ps_pool,
        ):
            # ---- persistent SBUF tensors ----
            wv_sb = cpool.tile([128, 8, 2 * S], bt, tag="wv")      # W_V row-chunks
            cg_sb = cpool.tile([128, 2, 2 * S], bt, tag="cg")
            co_sb = cpool.tile([128, 2, E], bt, tag="co")
            vb_sb = cpool.tile([128, 2 * S], dt, tag="vb")
            bo_sb = cpool.tile([128, E], dt, tag="bo")
            mwb_sb = cpool.tile([1, 2 * S], bt, tag="mwb")
            one_sb = cpool.tile([1, 1], dt, tag="one")
            one_bf = cpool.tile([1, 1], bt, tag="onebf")
            id_sb = cpool.tile([128, 128], dt, tag="ident")
            YT_sb = cpool.tile([128, T, 2], bt, tag="YT")          # y_t[c*128+p] at [p,t,c]
            R_row = cpool.tile([1, T], dt, tag="R")                # r_t at [0,t]
            RS_row = cpool.tile([1, T], bt, tag="RS")              # 1/r_t at [0,t]

            for c in range(8):
                nc.sync.dma_start(out=wv_sb[:, c, :], in_=wv_d[c * 128:(c + 1) * 128, :])
            for c in range(2):
                nc.sync.dma_start(out=cg_sb[:, c, :], in_=cg_d[c * 128:(c + 1) * 128, :])
                nc.sync.dma_start(out=co_sb[:, c, :], in_=co_d[c * 128:(c + 1) * 128, :])
            nc.sync.dma_start(out=vb_sb[:], in_=vb_d[:])
            nc.sync.dma_start(out=bo_sb[:], in_=bo_d[:])
            nc.sync.dma_start(out=mwb_sb[:], in_=mwb_d[:])
            nc.sync.dma_start(out=id_sb[:], in_=id_d[:])
            nc.vector.memset(one_sb[:], 1.0)
            nc.vector.memset(one_bf[:], 1.0)
            magic_sb = cpool.tile([1, 1], mybir.dt.uint32, tag="magic")
            nc.vector.memset(magic_sb[:], 0x5f3759df)
            half3_sb = cpool.tile([1, 1], dt, tag="half3")
            nc.vector.memset(half3_sb[:], 1.5)
            czero = cpool.tile([128, 1], dt, tag="czero")
            ceps = cpool.tile([128, 1], dt, tag="ceps")
            nc.vector.memset(czero[:], 0.0)
            nc.vector.memset(ceps[:], LN_EPS)
            nc.const_aps.aps[(dt, 0.0)] = czero[:]
            nc.const_aps.aps[(dt, LN_EPS)] = ceps[:]

            # ===== Phase 1: VT = (emb @ W_V + bias_v)^T, feature-major =====
            for i in range(NT):
                emb_t = embp.tile([128, E], dt, tag="emb")
                nc.sync.dma_start(out=emb_t[:], in_=emb_d[i * 128:(i + 1) * 128, :])
                embT = embtp.tile([128, 8, 128], bt, tag="embT")
                for c in range(8):
                    tp = tr_ps.tile([128, 128], dt, tag="tr")
                    nc.tensor.transpose(tp[:], emb_t[:, c * 128:(c + 1) * 128], id_sb[:])
                    nc.vector.tensor_copy(embT[:, c, :], tp[:])
                vps = big_ps.tile([128, 512], dt, tag="bigps")
                for ec in range(8):
                    nc.tensor.matmul(vps[:], lhsT=embT[:, ec, :], rhs=wv_sb[:, ec, :],
                                     start=(ec == 0), stop=(ec == 7))
                vrow_t = vstagep.tile([128, 2 * S], bt, tag="vstage")
                nc.vector.scalar_tensor_tensor(
                    out=vrow_t[:], in0=vps[:], scalar=1.0, in1=vb_sb[:],
                    op0=Alu.mult, op1=Alu.add)
                nc.sync.dma_start(out=v_dram[i * 128:(i + 1) * 128, :], in_=vrow_t[:])

            # ===== Phase 2: sequential scan =====
            # g_t (PSUM) = (1/r_{t-1}) * v_t + y_{t-1} @ Cg ; then
            # sigmoid/y-update apply r_{t-1} via the per-partition scale port.
            y_row = None
            if dbg_no_stats:
                nc.vector.memset(R_row[:], 1.0)
                nc.vector.memset(RS_row[:], 1.0)
            vbuf = None
            for t in range(T):
                if t % v_chunk == 0:
                    vbuf = vrowp.tile([1, v_chunk, 2 * S], bt, tag="vrow")
                    nc.sync.dma_start(out=vbuf[:], in_=v_dram[t:t + v_chunk, :])
                vsb = vbuf[0:1, t % v_chunk, :]
                if split_q:
                    qb = q_ps_pool.tile([1, S], dt, tag="qb")
                    qa = q_ps_pool.tile([1, S], dt, tag="qa")
                else:
                    qp = q_ps_pool.tile([1, 2 * S], dt, tag="qps")
                    qb, qa = qp[0:1, S:2 * S], qp[0:1, 0:S]
                if t == 0:
                    for dst, sl in ((qb, slice(S, 2 * S)), (qa, slice(0, S))):
                        nc.tensor.matmul(dst[:], lhsT=one_bf[0:1, 0:1], rhs=vsb[0:1, sl],
                                         start=True, stop=False)
                        nc.tensor.matmul(dst[:], lhsT=one_bf[0:1, 0:1], rhs=mwb_sb[0:1, sl],
                                         start=False, stop=True)
                    r_scale = 1.0
                else:
                    # transpose y_{t-1} into YT[:, t-1, :] via K=1 matmuls
                    if not dbg_no_tr:
                        ytp = yt_ps_pool.tile([128, 2], dt, tag="ytps")
                        for c in range(2):
                            nc.tensor.matmul(ytp[:, c:c + 1], lhsT=y_row[0:1, c * 128:(c + 1) * 128],
                                             rhs=one_sb[0:1, 0:1], start=True, stop=True)
                        nc.vector.tensor_copy(YT_sb[:, t - 1, :], ytp[:])
                    yt_col = 0 if dbg_no_tr else t - 1
                    # b-half first so sigmoid starts earliest; v-inject last so
                    # the RS (stats) wait overlaps the Cg matmuls
                    for dst, sl in ((qb, slice(S, 2 * S)), (qa, slice(0, S))):
                        for c in range(2):
                            nc.tensor.matmul(dst[:], lhsT=YT_sb[:, yt_col, c:c + 1],
                                             rhs=cg_sb[:, c, sl],
                                             start=(c == 0), stop=False)
                        nc.tensor.matmul(dst[:], lhsT=RS_row[0:1, t - 1:t], rhs=vsb[0:1, sl],
                                         start=False, stop=True)
                    r_scale = R_row[0:1, t - 1:t]

                sig = rowp.tile([1, S], dt, tag="sig")
                if dbg_no_act:
                    nc.vector.tensor_scalar(out=sig[:], in0=qb[:] if split_q else qb,
                                            scalar1=1.0, scalar2=None, op0=Alu.mult)
                else:
                    nc.scalar.activation(sig[:], qb[:] if split_q else qb, Act.Sigmoid, scale=r_scale)
                y_row = rowp.tile([1, S], dt, tag="y")
                sy = tiny.tile([1, 1], dt, tag="sy")
                nc.vector.scalar_tensor_tensor(
                    out=y_row[:], in0=qa[:] if split_q else qa, scalar=r_scale, in1=sig[:],
                    op0=Alu.mult, op1=Alu.mult, accum_out=sy[:])
                if dbg_no_stats:
                    continue
                # stats: Sq on ACT Square (same table as Sigmoid - no reload),
                # sy2/t2 + Newton-Raphson rsqrt (quake seed) on DVE
                ysq = rowp.tile([1, S], dt, tag="ysq")
                sq_acc = tiny.tile([1, 1], dt, tag="sq")
                nc.scalar.activation(ysq[:], y_row[:], Act.Square, scale=1.0 / 16.0,
                                     accum_out=sq_acc[:])
                sy2 = tiny.tile([1, 1], dt, tag="sy2")
                nc.vector.tensor_scalar(out=sy2[:], in0=sy[:], scalar1=sy[0:1, 0:1],
                                        scalar2=-1.0 / (S * S), op0=Alu.mult, op1=Alu.mult)
                t2 = tiny.tile([1, 1], dt, tag="t2")
                nc.vector.scalar_tensor_tensor(
                    out=t2[:], in0=sy2[:], scalar=LN_EPS, in1=sq_acc[:],
                    op0=Alu.add, op1=Alu.add)
                # r0 = quake seed; one NR iter: r = r0*(1.5 - 0.5*x*r0^2); RS = x*r
                sh = tiny.tile([1, 1], mybir.dt.uint32, tag="sh")
                nc.vector.tensor_scalar(out=sh[:], in0=t2[:].bitcast(mybir.dt.uint32),
                                        scalar1=1, scalar2=None, op0=Alu.logical_shift_right)
                seedi = tiny.tile([1, 1], mybir.dt.uint32, tag="seedi")
                nc.vector.tensor_tensor(out=seedi[:], in0=magic_sb[:], in1=sh[:], op=Alu.subtract)
                r0 = seedi[:].bitcast(dt)
                p0 = tiny.tile([1, 1], dt, tag="p0")
                nc.vector.tensor_scalar(out=p0[:], in0=r0, scalar1=r0, scalar2=-0.5,
                                        op0=Alu.mult, op1=Alu.mult)
                w0 = tiny.tile([1, 1], dt, tag="w0")
                nc.vector.scalar_tensor_tensor(
                    out=w0[:], in0=p0[:], scalar=t2[0:1, 0:1], in1=half3_sb[:],
                    op0=Alu.mult, op1=Alu.add)
                nc.vector.tensor_scalar(out=R_row[0:1, t:t + 1], in0=r0, scalar1=w0[0:1, 0:1],
                                        scalar2=None, op0=Alu.mult)
                nc.vector.tensor_scalar(out=RS_row[0:1, t:t + 1], in0=t2[:],
                                        scalar1=R_row[0:1, t:t + 1], scalar2=None, op0=Alu.mult)

            # final y_{T-1} transpose into YT
            ytp = yt_ps_pool.tile([128, 2], dt, tag="ytps")
            for c in range(2):
                nc.tensor.matmul(ytp[:, c:c + 1], lhsT=y_row[0:1, c * 128:(c + 1) * 128],
                                 rhs=one_sb[0:1, 0:1], start=True, stop=True)
            nc.vector.tensor_copy(YT_sb[:, T - 1, :], ytp[:])

            # ===== Phase 3: out = r * (y @ Co) + b_out =====
            for j in range(NT):
                rtp = yt_ps_pool.tile([128, 2], dt, tag="ytps")
                nc.tensor.matmul(rtp[:, 0:1], lhsT=R_row[0:1, j * 128:(j + 1) * 128],
                                 rhs=one_sb[0:1, 0:1], start=True, stop=True)
                rT = rowp.tile([128, 1], dt, tag="rT")
                nc.vector.tensor_copy(rT[:], rtp[:, 0:1])
                osb = outp.tile([128, E], dt, tag="osb")
                for nh in range(2):
                    ops = big_ps.tile([128, 512], dt, tag="bigps")
                    for c in range(2):
                        nc.tensor.matmul(
                            ops[:], lhsT=YT_sb[:, j * 128:(j + 1) * 128, c],
                            rhs=co_sb[:, c, nh * 512:(nh + 1) * 512],
                            start=(c == 0), stop=(c == 1))
                    nc.vector.scalar_tensor_tensor(
                        out=osb[:, nh * 512:(nh + 1) * 512], in0=ops[:],
                        scalar=rT[:, 0:1], in1=bo_sb[:, nh * 512:(nh + 1) * 512],
                        op0=Alu.mult, op1=Alu.add)
                nc.sync.dma_start(out=out_d[j * 128:(j + 1) * 128, :], in_=osb[:])

    nc.compile()
    return nc


_CACHE = {}


def _get_nc(T):
    if T not in _CACHE:
        _CACHE[T] = build_nc(T)
    return _CACHE[T]


def _run(inputs, trace=False):
    from concourse.bass_utils import run_bass_kernel_spmd

    emb = np.ascontiguousarray(inputs["embedded_tokens"], dtype=np.float32)
    B, T = emb.shape[0], emb.shape[1]
    folded = _fold_weights(inputs)
    nc = _get_nc(T)

    in_maps = []
    for b in range(B):
        m = {"emb": emb[b], "ident": np.eye(128, dtype=np.float32)}
        m.update(folded)
        in_maps.append(m)

    res = run_bass_kernel_spmd(nc, in_maps, core_ids=list(range(N_CORES)), trace=trace)
    out = np.stack([np.asarray(res.results[b]["out"]) for b in range(B)], axis=0)
    return out.astype(np.float32), res


def kernel(**inputs):
    out, _ = _run(inputs, trace=False)
    return out
